# revision 9
# baseline (speedup 1.0000x reference)
"""Two-layer dense-GAT forward on 8 Trainium2 NeuronCores.

Strategy (row-sharding per spec hint):
  - nodes are split into 8 blocks of 1024 rows; each core computes attention +
    aggregation for its row block against all 8192 columns.
  - e_ij = leakyrelu(src_i + dst_j) factorizes; softmax is computed
    unnormalized (exp without max-subtraction is safe for this data range) and
    the 0/1 adjacency is applied multiplicatively post-exp.  The softmax
    denominator rides the aggregation matmul as an appended ones-column.
  - relu(elu(x)) == relu(x) removes the layer-1 elu.
  - Three SPMD launches: (1) h1 = x@W1 (+src/dst heads) sharded fp32,
    (2) layer-1 attention + h2 = out1@W2 (+heads), (3) layer-2 attention + elu.
    The host gathers/reshards the small per-block results between launches.
"""

import sys

sys.path.insert(0, "/opt/trn_rl_repo")

import numpy as np
import ml_dtypes

import concourse.bass as bass
import concourse.mybir as mybir
import concourse.tile as tile
from concourse import bacc
from concourse.bass_utils import run_bass_kernel_spmd
from concourse.masks import make_identity

BF16 = ml_dtypes.bfloat16
F32 = mybir.dt.float32
F32R = mybir.dt.float32r
DBF = mybir.dt.bfloat16
AF = mybir.ActivationFunctionType
OP = mybir.AluOpType

N, FIN, H1, H2 = 8192, 512, 256, 128
NCORES = 8
R = N // NCORES          # rows per core
JC = N // 128            # 64 column chunks of 128
CG = 4                   # column chunks per processing group
NG = JC // CG            # 16 groups
ICN = R // 128           # 8 row chunks per core
FC1 = FIN // 128         # 4 contraction chunks for x@W1
ALPHA = 0.2
GRP_DVE = 4
EBUFS = 4               # every GRP_DVE-th group does leaky-relu on DVE instead of ACT

_cache: dict = {}


def _build_l1(reps=1):
    nc = bacc.Bacc("TRN2", target_bir_lowering=False, debug=False, num_devices=NCORES)
    xT_d = nc.dram_tensor("xT", [128, FC1, R], F32R, kind="ExternalInput")
    w_d = nc.dram_tensor("w1aug", [128, FC1, H1 + 2], F32R, kind="ExternalInput")
    o_d = nc.dram_tensor("h1sd", [ICN, 128, H1 + 2], F32, kind="ExternalOutput")
    with tile.TileContext(nc) as tc:
        with tc.tile_pool(name="sb", bufs=1) as sb, \
             tc.tile_pool(name="ps", bufs=1, space="PSUM") as ps, \
             tc.tile_pool(name="ob", bufs=2) as ob:
          for _rep in range(reps):
            xT = sb.tile([128, FC1, R], F32R, tag="xT", name="xT")
            w = sb.tile([128, FC1, H1 + 2], F32R, tag="w", name="w")
            nc.sync.dma_start(out=xT, in_=xT_d[:, :, :])
            nc.sync.dma_start(out=w, in_=w_d[:, :, :])
            pss = [ps.tile([128, H1 + 2], F32, tag=f"ps{i}", name=f"ps{i}") for i in range(ICN)]
            for fc in range(FC1):
                for i in range(ICN):
                    nc.tensor.matmul(pss[i], xT[:, fc, i * 128:(i + 1) * 128],
                                     w[:, fc, :],
                                     start=(fc == 0), stop=(fc == FC1 - 1))
            for i in range(ICN):
                o = ob.tile([128, H1 + 2], F32, tag="o", name="o")
                nc.vector.tensor_copy(o, pss[i])
                nc.sync.dma_start(out=o_d[i], in_=o)
    nc.compile()
    return nc


def _build_attn(layer, reps=1):
    """layer 1: F=H1 aggregate, tail computes h2/src2/dst2.
       layer 2: F=H2 aggregate, tail applies elu."""
    F = H1 if layer == 1 else H2
    FA = F + 1
    nc = bacc.Bacc("TRN2", target_bir_lowering=False, debug=False, num_devices=NCORES)
    src_d = nc.dram_tensor("srcb", [R], F32, kind="ExternalInput")
    dstT_d = nc.dram_tensor("dstT", [128, JC], F32, kind="ExternalInput")
    mask_d = nc.dram_tensor("mask", [128, JC, R], DBF, kind="ExternalInput")
    haug_d = nc.dram_tensor("haug", [128, JC, FA], DBF, kind="ExternalInput")
    if layer == 1:
        w2_d = nc.dram_tensor("w2aug", [128, H1 // 128, H2 + 2], F32,
                              kind="ExternalInput")
        o_d = nc.dram_tensor("h2sd", [ICN, 128, H2 + 2], F32, kind="ExternalOutput")
    else:
        o_d = nc.dram_tensor("out", [ICN, 128, H2], F32, kind="ExternalOutput")

    with tile.TileContext(nc) as tc:
        with tc.tile_pool(name="const", bufs=1) as cst, \
             tc.tile_pool(name="maskp", bufs=3) as maskp, \
             tc.tile_pool(name="ebuf", bufs=EBUFS) as ebuf, \
             tc.tile_pool(name="tbuf", bufs=2) as tbuf, \
             tc.tile_pool(name="pexp", bufs=2) as pexp, \
             tc.tile_pool(name="pfin", bufs=2) as pfin, \
             tc.tile_pool(name="smallp", bufs=2) as smallp, \
             tc.tile_pool(name="outp", bufs=2) as outp, \
             tc.tile_pool(name="psagg", bufs=1, space="PSUM") as psagg:
          for _rep in range(reps):
            warm = cst.tile([128, 1], F32, tag="warm", name="warm")
            nc.vector.memset(warm, 0.0)
            nc.scalar.activation(warm, warm, AF.Prelu, alpha=ALPHA)
            srcb = cst.tile([128, R], F32, tag="srcb", name="srcb")
            nc.sync.dma_start(out=srcb,
                              in_=bass.AP(tensor=src_d, offset=0,
                                          ap=[[0, 128], [1, R]]))
            dstT = cst.tile([128, JC], F32, tag="dstT")
            nc.sync.dma_start(out=dstT, in_=dstT_d[:, :])
            haug = cst.tile([128, JC, FA], DBF, tag="haug")
            nc.sync.dma_start(out=haug, in_=haug_d[:, :, :])
            if layer == 1:
                w2 = cst.tile([128, H1 // 128, H2 + 2], F32, tag="w2")
                nc.sync.dma_start(out=w2, in_=w2_d[:, :, :])
                ident = cst.tile([128, 128], F32, tag="ident")
                make_identity(nc, ident)

            agg = [psagg.tile([128, FA], F32, tag=f"agg{i}", name=f"agg{i}") for i in range(ICN)]
            for g in range(NG):
                M = maskp.tile([128, CG, R], DBF, tag="M")
                nc.sync.dma_start(out=M, in_=mask_d[:, g * CG:(g + 1) * CG, :])
                E = ebuf.tile([128, CG, R], F32, tag="E")
                for c in range(CG):
                    jc = g * CG + c
                    nc.vector.tensor_scalar_add(E[:, c, :], srcb,
                                                dstT[:, jc:jc + 1])
                if g % GRP_DVE == GRP_DVE - 1:
                    # leaky-relu on DVE: E = max(E, 0.2*E)
                    U = tbuf.tile([128, CG, R], F32, tag="U", bufs=1)
                    nc.vector.tensor_scalar_mul(U, E, ALPHA)
                    nc.vector.tensor_tensor(E, E, U, OP.max)
                else:
                    nc.scalar.activation(E, E, AF.Prelu, alpha=ALPHA)
                PX = pexp.tile([128, CG, R], DBF, tag="PX")
                nc.scalar.activation(PX, E, AF.Exp)
                PF = pfin.tile([128, CG, R], DBF, tag="PF")
                nc.vector.tensor_tensor(PF, PX, M, OP.mult)
                for c in range(CG):
                    jc = g * CG + c
                    for i in range(ICN):
                        nc.tensor.matmul(agg[i], PF[:, c, i * 128:(i + 1) * 128],
                                         haug[:, jc, :],
                                         start=(jc == 0), stop=(jc == JC - 1))

            if layer == 1:
                o1T = cst.tile([128, H1 // 128, R], F32, tag="o1T")
                for i in range(ICN):
                    r = smallp.tile([128, 1], F32, tag="r")
                    nc.vector.reciprocal(r, agg[i][:, F:F + 1])
                    o1 = outp.tile([128, F], F32, tag=f"o1_{i}", bufs=1)
                    nc.vector.tensor_scalar(o1, agg[i][:, 0:F], r[:, :], 0.0,
                                            OP.mult, OP.max)
                    for fcc in range(H1 // 128):
                        tp = psagg.tile([128, 128], F32, tag=f"agg{i}")
                        nc.tensor.transpose(tp, o1[:, fcc * 128:(fcc + 1) * 128],
                                            ident)
                        nc.vector.tensor_copy(o1T[:, fcc, i * 128:(i + 1) * 128], tp)
                for i in range(ICN):
                    h2ps = psagg.tile([128, H2 + 2], F32, tag=f"agg{i}")
                    for fcc in range(H1 // 128):
                        nc.tensor.matmul(h2ps, o1T[:, fcc, i * 128:(i + 1) * 128],
                                         w2[:, fcc, :],
                                         start=(fcc == 0),
                                         stop=(fcc == H1 // 128 - 1))
                    ho = outp.tile([128, H2 + 2], F32, tag="ho")
                    nc.vector.tensor_copy(ho, h2ps)
                    nc.sync.dma_start(out=o_d[i], in_=ho)
            else:
                for i in range(ICN):
                    r = smallp.tile([128, 1], F32, tag="r")
                    nc.vector.reciprocal(r, agg[i][:, F:F + 1])
                    # elu(x) = relu(x) + exp(min(x, 0)) - 1, with x = agg/rowsum
                    xn = smallp.tile([128, H2], F32, tag="xn")
                    nc.vector.tensor_scalar(xn, agg[i][:, 0:F], r[:, :], 0.0,
                                            OP.mult, OP.min)
                    xp = smallp.tile([128, H2], F32, tag="xp")
                    nc.vector.tensor_scalar(xp, agg[i][:, 0:F], r[:, :], 0.0,
                                            OP.mult, OP.max)
                    xe = smallp.tile([128, H2], F32, tag="xe")
                    nc.scalar.activation(xe, xn, AF.Exp)
                    oo = outp.tile([128, H2], F32, tag="oo")
                    nc.vector.tensor_tensor(oo, xp, xe, OP.add)
                    nc.vector.tensor_scalar_add(oo, oo, -1.0)
                    nc.sync.dma_start(out=o_d[i], in_=oo)
    nc.compile()
    return nc


def _get(name, builder):
    if name not in _cache:
        _cache[name] = builder()
    return _cache[name]


def _prep_host(x, adj, W1, a1, W2, a2):
    x = np.asarray(x, np.float32)
    W1 = np.asarray(W1, np.float32)
    a1 = np.asarray(a1, np.float32)
    W2 = np.asarray(W2, np.float32)
    a2 = np.asarray(a2, np.float32)

    w1aug = np.concatenate([W1, W1 @ a1[:H1], W1 @ a1[H1:]], axis=1)  # [512,258]
    w1aug = np.ascontiguousarray(
        w1aug.reshape(FC1, 128, H1 + 2).transpose(1, 0, 2))
    w2aug = np.concatenate([W2, W2 @ a2[:H2], W2 @ a2[H2:]], axis=1)  # [256,130]
    w2aug = np.ascontiguousarray(
        w2aug.reshape(H1 // 128, 128, H2 + 2).transpose(1, 0, 2))

    adjT = np.asarray(adj).T.astype(BF16)  # [N, N] column-major mask view
    masks = []
    xTs = []
    for c in range(NCORES):
        blk = slice(c * R, (c + 1) * R)
        mc = adjT[:, blk].reshape(JC, 128, R).transpose(1, 0, 2)
        masks.append(np.ascontiguousarray(mc))
        xt = x[blk].T.reshape(FC1, 128, R).transpose(1, 0, 2)
        xTs.append(np.ascontiguousarray(xt))
    return xTs, w1aug, w2aug, masks


def _haug(h, F):
    """[N, F] fp32 -> [128, JC, F+1] bf16 with ones column."""
    hb = h.reshape(JC, 128, F).transpose(1, 0, 2).astype(BF16)
    ones = np.ones((128, JC, 1), BF16)
    return np.ascontiguousarray(np.concatenate([hb, ones], axis=2))


def _dstT(d):
    return np.ascontiguousarray(d.reshape(JC, 128).T.astype(np.float32))


def kernel(x, adj, W1, a1, W2, a2):
    xTs, w1aug, w2aug, masks = _prep_host(x, adj, W1, a1, W2, a2)
    cores = list(range(NCORES))

    nc1 = _get("l1", _build_l1)
    res1 = run_bass_kernel_spmd(
        nc1, [dict(xT=xTs[c], w1aug=w1aug) for c in cores], cores)
    h1sd = np.concatenate(
        [res1.results[c]["h1sd"].reshape(R, H1 + 2) for c in cores])  # [N, 258]
    h1 = h1sd[:, :H1]
    src1 = h1sd[:, H1]
    dst1 = h1sd[:, H1 + 1]

    haug1 = _haug(h1, H1)
    dstT1 = _dstT(dst1)
    nc2 = _get("attn1", lambda: _build_attn(1))
    res2 = run_bass_kernel_spmd(
        nc2,
        [dict(srcb=np.ascontiguousarray(src1[c * R:(c + 1) * R]),
              dstT=dstT1, mask=masks[c], haug=haug1, w2aug=w2aug)
         for c in cores],
        cores)
    h2sd = np.concatenate(
        [res2.results[c]["h2sd"].reshape(R, H2 + 2) for c in cores])  # [N, 130]
    h2 = h2sd[:, :H2]
    src2 = h2sd[:, H2]
    dst2 = h2sd[:, H2 + 1]

    haug2 = _haug(h2, H2)
    dstT2 = _dstT(dst2)
    nc3 = _get("attn2", lambda: _build_attn(2))
    res3 = run_bass_kernel_spmd(
        nc3,
        [dict(srcb=np.ascontiguousarray(src2[c * R:(c + 1) * R]),
              dstT=dstT2, mask=masks[c], haug=haug2)
         for c in cores],
        cores)
    out = np.concatenate(
        [res3.results[c]["out"].reshape(R, H2) for c in cores])
    return out.astype(np.float32)


# revision 11
# speedup vs baseline: 84.5694x; 84.5694x over previous
"""Two-layer dense-GAT forward on 8 Trainium2 NeuronCores.

Strategy (row-sharding per spec hint):
  - nodes are split into 8 blocks of 1024 rows; each core computes attention +
    aggregation for its row block against all 8192 columns.
  - e_ij = leakyrelu(src_i + dst_j) factorizes; softmax is computed
    unnormalized (exp without max-subtraction is safe for this data range) and
    the 0/1 adjacency is applied multiplicatively post-exp.  The softmax
    denominator rides the aggregation matmul as an appended ones-column.
  - relu(elu(x)) == relu(x) removes the layer-1 elu.
  - Three SPMD launches: (1) h1 = x@W1 (+src/dst heads) sharded fp32,
    (2) layer-1 attention + h2 = out1@W2 (+heads), (3) layer-2 attention + elu.
    The host gathers/reshards the small per-block results between launches.
"""

import sys

sys.path.insert(0, "/opt/trn_rl_repo")

import numpy as np
import ml_dtypes

import concourse.bass as bass
import concourse.mybir as mybir
import concourse.tile as tile
from concourse import bacc
from concourse.bass_utils import run_bass_kernel_spmd
from concourse.masks import make_identity

BF16 = ml_dtypes.bfloat16
F32 = mybir.dt.float32
F32R = mybir.dt.float32r
DBF = mybir.dt.bfloat16
AF = mybir.ActivationFunctionType
OP = mybir.AluOpType

N, FIN, H1, H2 = 8192, 512, 256, 128
NCORES = 8
R = N // NCORES          # rows per core
JC = N // 128            # 64 column chunks of 128
CG = 4                   # column chunks per processing group
NG = JC // CG            # 16 groups
ICN = R // 128           # 8 row chunks per core
FC1 = FIN // 128         # 4 contraction chunks for x@W1
ALPHA = 0.2
GRP_DVE = 4
EBUFS = 4               # every GRP_DVE-th group does leaky-relu on DVE instead of ACT

_cache: dict = {}


def _build_l1(reps=1):
    nc = bacc.Bacc("TRN2", target_bir_lowering=False, debug=False, num_devices=NCORES)
    xT_d = nc.dram_tensor("xT", [128, FC1, R], F32R, kind="ExternalInput")
    w_d = nc.dram_tensor("w1aug", [128, FC1, H1 + 2], F32R, kind="ExternalInput")
    o_d = nc.dram_tensor("h1sd", [ICN, 128, H1 + 2], F32, kind="ExternalOutput")
    with tile.TileContext(nc) as tc:
        with tc.tile_pool(name="sb", bufs=1) as sb, \
             tc.tile_pool(name="ps", bufs=1, space="PSUM") as ps, \
             tc.tile_pool(name="ob", bufs=2) as ob:
          for _rep in range(reps):
            xT = sb.tile([128, FC1, R], F32R, tag="xT", name="xT")
            w = sb.tile([128, FC1, H1 + 2], F32R, tag="w", name="w")
            nc.sync.dma_start(out=xT, in_=xT_d[:, :, :])
            nc.sync.dma_start(out=w, in_=w_d[:, :, :])
            pss = [ps.tile([128, H1 + 2], F32, tag=f"ps{i}", name=f"ps{i}") for i in range(ICN)]
            for fc in range(FC1):
                for i in range(ICN):
                    nc.tensor.matmul(pss[i], xT[:, fc, i * 128:(i + 1) * 128],
                                     w[:, fc, :],
                                     start=(fc == 0), stop=(fc == FC1 - 1))
            for i in range(ICN):
                o = ob.tile([128, H1 + 2], F32, tag="o", name="o")
                nc.vector.tensor_copy(o, pss[i])
                nc.sync.dma_start(out=o_d[i], in_=o)
    nc.compile()
    return nc


def _build_attn(layer, reps=1):
    """layer 1: F=H1 aggregate, tail computes h2/src2/dst2.
       layer 2: F=H2 aggregate, tail applies elu."""
    F = H1 if layer == 1 else H2
    FA = F + 1
    nc = bacc.Bacc("TRN2", target_bir_lowering=False, debug=False, num_devices=NCORES)
    src_d = nc.dram_tensor("srcb", [R], F32, kind="ExternalInput")
    dstT_d = nc.dram_tensor("dstT", [128, JC], F32, kind="ExternalInput")
    mask_d = nc.dram_tensor("mask", [128, JC, R], DBF, kind="ExternalInput")
    haug_d = nc.dram_tensor("haug", [128, JC, FA], DBF, kind="ExternalInput")
    if layer == 1:
        w2_d = nc.dram_tensor("w2aug", [128, H1 // 128, H2 + 2], F32,
                              kind="ExternalInput")
        o_d = nc.dram_tensor("h2sd", [ICN, 128, H2 + 2], F32, kind="ExternalOutput")
    else:
        o_d = nc.dram_tensor("out", [ICN, 128, H2], F32, kind="ExternalOutput")

    with tile.TileContext(nc) as tc:
        with tc.tile_pool(name="const", bufs=1) as cst, \
             tc.tile_pool(name="maskp", bufs=3) as maskp, \
             tc.tile_pool(name="ebuf", bufs=EBUFS) as ebuf, \
             tc.tile_pool(name="tbuf", bufs=2) as tbuf, \
             tc.tile_pool(name="pexp", bufs=2) as pexp, \
             tc.tile_pool(name="pfin", bufs=3) as pfin, \
             tc.tile_pool(name="smallp", bufs=2) as smallp, \
             tc.tile_pool(name="outp", bufs=2) as outp, \
             tc.tile_pool(name="psagg", bufs=1, space="PSUM") as psagg:
          for _rep in range(reps):
            warm = cst.tile([128, 1], F32, tag="warm", name="warm")
            nc.vector.memset(warm, 0.0)
            nc.scalar.activation(warm, warm, AF.Prelu, alpha=ALPHA)
            srcb = cst.tile([128, R], F32, tag="srcb", name="srcb")
            nc.sync.dma_start(out=srcb,
                              in_=bass.AP(tensor=src_d, offset=0,
                                          ap=[[0, 128], [1, R]]))
            dstT = cst.tile([128, JC], F32, tag="dstT")
            nc.sync.dma_start(out=dstT, in_=dstT_d[:, :])
            haug = cst.tile([128, JC, FA], DBF, tag="haug")
            nc.sync.dma_start(out=haug, in_=haug_d[:, :, :])
            if layer == 1:
                w2 = cst.tile([128, H1 // 128, H2 + 2], F32, tag="w2")
                nc.sync.dma_start(out=w2, in_=w2_d[:, :, :])
                ident = cst.tile([128, 128], F32, tag="ident")
                make_identity(nc, ident)

            agg = [psagg.tile([128, FA], F32, tag=f"agg{i}", name=f"agg{i}") for i in range(ICN)]
            for g in range(NG):
                M = maskp.tile([128, CG, R], DBF, tag="M")
                nc.sync.dma_start(out=M, in_=mask_d[:, g * CG:(g + 1) * CG, :])
                E = ebuf.tile([128, CG, R], F32, tag="E")
                for c in range(CG):
                    jc = g * CG + c
                    nc.vector.tensor_scalar_add(E[:, c, :], srcb,
                                                dstT[:, jc:jc + 1])
                if g % GRP_DVE == GRP_DVE - 1:
                    # leaky-relu on DVE: E = max(E, 0.2*E)
                    U = tbuf.tile([128, CG, R], F32, tag="U", bufs=1)
                    nc.vector.tensor_scalar_mul(U, E, ALPHA)
                    nc.vector.tensor_tensor(E, E, U, OP.max)
                else:
                    nc.scalar.activation(E, E, AF.Prelu, alpha=ALPHA)
                PX = pexp.tile([128, CG, R], DBF, tag="PX")
                nc.scalar.activation(PX, E, AF.Exp)
                PF = pfin.tile([128, CG, R], DBF, tag="PF")
                nc.vector.tensor_tensor(PF, PX, M, OP.mult)
                for c in range(CG):
                    jc = g * CG + c
                    for i in range(ICN):
                        nc.tensor.matmul(agg[i], PF[:, c, i * 128:(i + 1) * 128],
                                         haug[:, jc, :],
                                         start=(jc == 0), stop=(jc == JC - 1))

            if layer == 1:
                o1T = cst.tile([128, H1 // 128, R], F32, tag="o1T")
                for i in range(ICN):
                    r = smallp.tile([128, 1], F32, tag="r")
                    nc.vector.reciprocal(r, agg[i][:, F:F + 1])
                    o1 = outp.tile([128, F], F32, tag=f"o1_{i}", bufs=1)
                    nc.vector.tensor_scalar(o1, agg[i][:, 0:F], r[:, :], 0.0,
                                            OP.mult, OP.max)
                    for fcc in range(H1 // 128):
                        tp = psagg.tile([128, 128], F32, tag=f"agg{i}")
                        nc.tensor.transpose(tp, o1[:, fcc * 128:(fcc + 1) * 128],
                                            ident)
                        nc.vector.tensor_copy(o1T[:, fcc, i * 128:(i + 1) * 128], tp)
                for i in range(ICN):
                    h2ps = psagg.tile([128, H2 + 2], F32, tag=f"agg{i}")
                    for fcc in range(H1 // 128):
                        nc.tensor.matmul(h2ps, o1T[:, fcc, i * 128:(i + 1) * 128],
                                         w2[:, fcc, :],
                                         start=(fcc == 0),
                                         stop=(fcc == H1 // 128 - 1))
                    ho = outp.tile([128, H2 + 2], F32, tag="ho")
                    nc.vector.tensor_copy(ho, h2ps)
                    nc.sync.dma_start(out=o_d[i], in_=ho)
            else:
                for i in range(ICN):
                    r = smallp.tile([128, 1], F32, tag="r")
                    nc.vector.reciprocal(r, agg[i][:, F:F + 1])
                    # elu(x) = relu(x) + exp(min(x, 0)) - 1, with x = agg/rowsum
                    xn = smallp.tile([128, H2], F32, tag="xn")
                    nc.vector.tensor_scalar(xn, agg[i][:, 0:F], r[:, :], 0.0,
                                            OP.mult, OP.min)
                    xp = smallp.tile([128, H2], F32, tag="xp")
                    nc.vector.tensor_scalar(xp, agg[i][:, 0:F], r[:, :], 0.0,
                                            OP.mult, OP.max)
                    xe = smallp.tile([128, H2], F32, tag="xe")
                    nc.scalar.activation(xe, xn, AF.Exp)
                    oo = outp.tile([128, H2], F32, tag="oo")
                    nc.vector.tensor_tensor(oo, xp, xe, OP.add)
                    nc.vector.tensor_scalar_add(oo, oo, -1.0)
                    nc.sync.dma_start(out=o_d[i], in_=oo)
    nc.compile()
    return nc


def _get(name, builder):
    if name not in _cache:
        _cache[name] = builder()
    return _cache[name]


def _prep_host(x, adj, W1, a1, W2, a2):
    x = np.asarray(x, np.float32)
    W1 = np.asarray(W1, np.float32)
    a1 = np.asarray(a1, np.float32)
    W2 = np.asarray(W2, np.float32)
    a2 = np.asarray(a2, np.float32)

    w1aug = np.concatenate([W1, W1 @ a1[:H1], W1 @ a1[H1:]], axis=1)  # [512,258]
    w1aug = np.ascontiguousarray(
        w1aug.reshape(FC1, 128, H1 + 2).transpose(1, 0, 2))
    w2aug = np.concatenate([W2, W2 @ a2[:H2], W2 @ a2[H2:]], axis=1)  # [256,130]
    w2aug = np.ascontiguousarray(
        w2aug.reshape(H1 // 128, 128, H2 + 2).transpose(1, 0, 2))

    adjT = np.asarray(adj).T.astype(BF16)  # [N, N] column-major mask view
    masks = []
    xTs = []
    for c in range(NCORES):
        blk = slice(c * R, (c + 1) * R)
        mc = adjT[:, blk].reshape(JC, 128, R).transpose(1, 0, 2)
        masks.append(np.ascontiguousarray(mc))
        xt = x[blk].T.reshape(FC1, 128, R).transpose(1, 0, 2)
        xTs.append(np.ascontiguousarray(xt))
    return xTs, w1aug, w2aug, masks


def _haug(h, F):
    """[N, F] fp32 -> [128, JC, F+1] bf16 with ones column."""
    hb = h.reshape(JC, 128, F).transpose(1, 0, 2).astype(BF16)
    ones = np.ones((128, JC, 1), BF16)
    return np.ascontiguousarray(np.concatenate([hb, ones], axis=2))


def _dstT(d):
    return np.ascontiguousarray(d.reshape(JC, 128).T.astype(np.float32))


def _run(nc, in_maps, cores):
    """run_bass_kernel_spmd with one retry (transient device errors)."""
    try:
        return run_bass_kernel_spmd(nc, in_maps, cores)
    except Exception:
        return run_bass_kernel_spmd(nc, in_maps, cores)


def kernel(x, adj, W1, a1, W2, a2):
    xTs, w1aug, w2aug, masks = _prep_host(x, adj, W1, a1, W2, a2)
    cores = list(range(NCORES))

    nc1 = _get("l1", _build_l1)
    res1 = _run(nc1, [dict(xT=xTs[c], w1aug=w1aug) for c in cores], cores)
    h1sd = np.concatenate(
        [res1.results[c]["h1sd"].reshape(R, H1 + 2) for c in cores])  # [N, 258]
    h1 = h1sd[:, :H1]
    src1 = h1sd[:, H1]
    dst1 = h1sd[:, H1 + 1]

    haug1 = _haug(h1, H1)
    dstT1 = _dstT(dst1)
    nc2 = _get("attn1", lambda: _build_attn(1))
    res2 = _run(
        nc2,
        [dict(srcb=np.ascontiguousarray(src1[c * R:(c + 1) * R]),
              dstT=dstT1, mask=masks[c], haug=haug1, w2aug=w2aug)
         for c in cores],
        cores)
    h2sd = np.concatenate(
        [res2.results[c]["h2sd"].reshape(R, H2 + 2) for c in cores])  # [N, 130]
    h2 = h2sd[:, :H2]
    src2 = h2sd[:, H2]
    dst2 = h2sd[:, H2 + 1]

    haug2 = _haug(h2, H2)
    dstT2 = _dstT(dst2)
    nc3 = _get("attn2", lambda: _build_attn(2))
    res3 = _run(
        nc3,
        [dict(srcb=np.ascontiguousarray(src2[c * R:(c + 1) * R]),
              dstT=dstT2, mask=masks[c], haug=haug2)
         for c in cores],
        cores)
    out = np.concatenate(
        [res3.results[c]["out"].reshape(R, H2) for c in cores])
    return out.astype(np.float32)
